# revision 3
# baseline (speedup 1.0000x reference)
"""Trainium2 Bass kernel for batched cross-attention with gaussian guide mask.

Reference computation (per batch b):
  Q   = query @ Wq.T                      # [Tq, A]
  att = (Q @ K.T / sqrt(A)) * guide       # guide[n] = exp(-(step-(n+1)/N)^2/TEMP)
  att = where(mask, -inf, att)
  out = softmax(att, axis=-1) @ V         # [Tq, E]

Sharding: data-parallel over batch. Core b handles batch b (B == 8 == n_cores).

Device-side layout choices (host does layout-only prep: transposes/casts):
  qT  = query[b].T   [L, Tq]   - so Q^T comes out of PE with A on partitions
  kT  = K[b].T       [A, N]    - guide and 1/sqrt(A) folded in on device
  v   = V[b]         [N, E]    - natural; AV contraction tiles n on partitions
  wqT = Wq.T         [L, A]
  msk = mask[b] u8   [Tq, N]
Softmax is computed without max-subtraction (att values are O(5), exp is safe
in f32, and softmax is shift-invariant); masked lanes are set to -200 before
exp so exp underflows to 0 and the fused accumulate row-sum is exact.
Normalization is applied to the [128, 512] output tile instead of the
[128, 2048] score tile (linearity of the AV matmul).
"""

import math

import numpy as np

import concourse.bass as bass
import concourse.mybir as mybir
import concourse.tile as tile
from concourse import bacc
from concourse.bass import ts
from concourse.bass_utils import run_bass_kernel_spmd
from concourse.masks import make_identity

B, TQ, N = 8, 1024, 2048
L, A, E = 1024, 128, 512
TEMP = 0.08
P = 128
LT = L // P    # 8 l-tiles (contraction tiles of the Q projection)
TT = TQ // P   # 8 t-tiles (rows of attention, 128 at a time)
NT = N // P    # 16 n-tiles (contraction tiles of the AV matmul)
NEG = -200.0   # masked logit value; exp(-200) underflows to exactly 0 in f32

F32 = mybir.dt.float32
U8 = mybir.dt.uint8


def build_nc():
    nc = bacc.Bacc("TRN2", target_bir_lowering=False, debug=False, num_devices=B)

    qT = nc.dram_tensor("qT", [L, TQ], F32, kind="ExternalInput").ap()
    kT = nc.dram_tensor("kT", [A, N], F32, kind="ExternalInput").ap()
    v = nc.dram_tensor("v", [N, E], F32, kind="ExternalInput").ap()
    wqT = nc.dram_tensor("wqT", [L, A], F32, kind="ExternalInput").ap()
    stp = nc.dram_tensor("stp", [1, 1], F32, kind="ExternalInput").ap()
    msk = nc.dram_tensor("msk", [TQ, N], U8, kind="ExternalInput").ap()
    out = nc.dram_tensor("out", [TQ, E], F32, kind="ExternalOutput").ap()

    with tile.TileContext(nc) as tc:
        with (
            tc.tile_pool(name="const", bufs=1) as const,
            tc.tile_pool(name="setup", bufs=1) as setup,
            tc.tile_pool(name="mpool", bufs=3) as mpool,
            tc.tile_pool(name="spool", bufs=2) as spool,
            tc.tile_pool(name="stpool", bufs=2) as stpool,
            tc.tile_pool(name="opool", bufs=3) as opool,
            tc.tile_pool(name="small", bufs=6) as small,
            tc.tile_pool(name="psA", bufs=1, space="PSUM") as psA,
            tc.tile_pool(name="psB", bufs=2, space="PSUM") as psB,
            tc.tile_pool(name="psO", bufs=2, space="PSUM") as psO,
        ):
            # ---- one-time setup ----
            ident = const.tile([P, P], F32)
            make_identity(nc, ident)

            step_sb = const.tile([P, 1], F32)
            nc.gpsimd.dma_start(out=step_sb, in_=stp.to_broadcast((P, 1)))
            nstep = const.tile([P, 1], F32)
            nc.vector.tensor_scalar_mul(nstep, step_sb, -1.0)

            neg_tile = const.tile([P, N], F32)
            nc.vector.memset(neg_tile, NEG)

            # guide row, replicated across all 128 partitions:
            #   guide[n] = exp(-((n+1)/N - step)^2 / TEMP - 0.5*ln(A))
            # (the 1/sqrt(A) attention norm is folded into the bias)
            pos = setup.tile([P, N], F32)
            nc.gpsimd.iota(
                pos,
                pattern=[[1, N]],
                base=1,
                channel_multiplier=0,
                allow_small_or_imprecise_dtypes=True,
            )
            z = setup.tile([P, N], F32)
            nc.scalar.activation(
                out=z,
                in_=pos,
                func=mybir.ActivationFunctionType.Square,
                bias=nstep,
                scale=1.0 / N,
            )
            gbias = const.tile([P, 1], F32)
            nc.vector.memset(gbias, -0.5 * math.log(A))
            guide = setup.tile([P, N], F32)
            nc.scalar.activation(
                out=guide,
                in_=z,
                func=mybir.ActivationFunctionType.Exp,
                scale=-1.0 / TEMP,
                bias=gbias,
            )

            # K^T with guide folded in: ksc[a, n] = K[n, a] * guide[n]
            kt_sb = setup.tile([P, N], F32)
            nc.sync.dma_start(out=kt_sb, in_=kT)
            ksc = const.tile([P, N], F32)
            nc.vector.tensor_mul(ksc, kt_sb, guide)

            # Wq^T tiles: wq_sb[p, lt, a] = Wq[a, lt*128+p]
            wq_sb = const.tile([P, LT, A], F32)
            nc.sync.dma_start(out=wq_sb, in_=wqT.rearrange("(lt p) a -> p lt a", p=P))

            # query^T tiles: qt_in[p, lt, t] = query[t, lt*128+p]
            # (split the DMA by l-tile so Q-projection matmuls start early)
            qt_in = setup.tile([P, LT, TQ], F32)
            for lt in range(LT):
                nc.sync.dma_start(
                    out=qt_in[:, lt, :],
                    in_=qT.rearrange("(lt p) t -> p lt t", p=P)[:, lt, :],
                )

            # Q^T[a, t] = sum_l Wq[a, l] * query[t, l]
            qt = const.tile([P, TQ], F32)
            for th in range(TQ // 512):
                ps_qt = psB.tile([P, 512], F32, tag="psb")
                for lt in range(LT):
                    nc.tensor.matmul(
                        ps_qt,
                        wq_sb[:, lt, :],
                        qt_in[:, lt, ts(th, 512)],
                        start=(lt == 0),
                        stop=(lt == LT - 1),
                    )
                nc.scalar.copy(qt[:, ts(th, 512)], ps_qt)

            # V tiles: v_sb[p, nt, e] = V[nt*128+p, e]
            v_sb = const.tile([P, NT, E], F32)
            for vh in range(4):
                nc.sync.dma_start(
                    out=v_sb[:, ts(vh, NT // 4), :],
                    in_=v.rearrange("(nt p) e -> p nt e", p=P)[:, ts(vh, NT // 4), :],
                )

            # ---- main loop over 128-row tiles of Tq ----
            for ti in range(TT):
                mk = mpool.tile([P, N], U8)
                nc.sync.dma_start(out=mk, in_=msk[ts(ti, P), :])

                # att[t, n] = Q^T[:, t-tile].T @ ksc  (guide and norm already in ksc)
                att = psA.tile([P, N], F32)
                for j in range(N // 512):
                    nc.tensor.matmul(
                        att[:, ts(j, 512)],
                        qt[:, ts(ti, P)],
                        ksc[:, ts(j, 512)],
                        start=True,
                        stop=True,
                    )

                # masked lanes -> -200 (exp underflows to 0)
                nc.vector.copy_predicated(out=att, mask=mk, data=neg_tile)

                # s = exp(att), rs = row-sum(s) fused on the scalar engine
                s = spool.tile([P, N], F32)
                rs = small.tile([P, 1], F32)
                nc.scalar.activation(
                    out=s,
                    in_=att,
                    func=mybir.ActivationFunctionType.Exp,
                    accum_out=rs,
                )
                rc = small.tile([P, 1], F32)
                nc.vector.reciprocal(rc, rs)

                # s^T in 128x128 blocks via PE transpose (4 blocks per PSUM bank)
                st = stpool.tile([P, N], F32)
                for g in range(4):
                    ps_tr = psB.tile([P, 512], F32, tag="psb")
                    for j in range(4):
                        nc.tensor.transpose(
                            ps_tr[:, ts(j, P)], s[:, ts(g * 4 + j, P)], ident
                        )
                    if g % 2 == 0:
                        nc.scalar.copy(st[:, ts(g, 512)], ps_tr)
                    else:
                        nc.vector.tensor_copy(st[:, ts(g, 512)], ps_tr)

                # out[t, e] = sum_n s[t, n] * V[n, e], accumulated over 16 n-tiles
                ot = psO.tile([P, E], F32)
                for nt in range(NT):
                    nc.tensor.matmul(
                        ot,
                        st[:, ts(nt, P)],
                        v_sb[:, nt, :],
                        start=(nt == 0),
                        stop=(nt == NT - 1),
                    )

                # normalize on the narrow output tile and store
                ob = opool.tile([P, E], F32)
                nc.scalar.activation(
                    out=ob,
                    in_=ot,
                    func=mybir.ActivationFunctionType.Copy,
                    scale=rc,
                )
                nc.gpsimd.dma_start(out=out[ts(ti, P), :], in_=ob)

    nc.compile()
    return nc


def make_in_maps(query, K, V, Wq, step, mask):
    query = np.asarray(query, dtype=np.float32)
    K = np.asarray(K, dtype=np.float32)
    V = np.asarray(V, dtype=np.float32)
    Wq = np.asarray(Wq, dtype=np.float32)
    step = np.asarray(step, dtype=np.float32)
    mask = np.asarray(mask)
    if mask.dtype != np.uint8:
        mask = mask.astype(np.uint8)

    wqT = np.ascontiguousarray(Wq.T)
    stp = step.reshape(1, 1)
    in_maps = []
    for b in range(B):
        in_maps.append(
            {
                "qT": np.ascontiguousarray(query[b].T),
                "kT": np.ascontiguousarray(K[b].T),
                "v": np.ascontiguousarray(V[b]),
                "wqT": wqT,
                "stp": stp,
                "msk": mask[b],
            }
        )
    return in_maps


def kernel(query, K, V, Wq, step, mask):
    nc = build_nc()
    in_maps = make_in_maps(query, K, V, Wq, step, mask)
    res = run_bass_kernel_spmd(nc, in_maps, core_ids=list(range(B)))
    return np.stack([res.results[b]["out"] for b in range(B)], axis=0)


if __name__ == "__main__":
    rng = np.random.default_rng(0)
    inputs = {
        "query": rng.standard_normal((B, TQ, L), dtype=np.float32),
        "K": rng.standard_normal((B, N, A), dtype=np.float32),
        "V": rng.standard_normal((B, N, E), dtype=np.float32),
        "Wq": rng.standard_normal((A, L), dtype=np.float32) / math.sqrt(L),
        "step": rng.random((1,), dtype=np.float32),
        "mask": rng.integers(0, 2, size=(B, TQ, N)) > 0,
    }
    out = kernel(**inputs)
    print(out.shape, out.dtype)


# revision 7
# speedup vs baseline: 1.6391x; 1.6391x over previous
"""Trainium2 Bass kernel for batched cross-attention with gaussian guide mask.

Reference computation (per batch b):
  Q   = query @ Wq.T                      # [Tq, A]
  att = (Q @ K.T / sqrt(A)) * guide       # guide[n] = exp(-(step-(n+1)/N)^2/TEMP)
  att = where(mask, -inf, att)
  out = softmax(att, axis=-1) @ V         # [Tq, E]

Sharding: data-parallel over batch. Core b handles batch b (B == 8 == n_cores).

Device-side layout choices (host does layout-only prep: transposes/casts):
  qT  = query[b].T   [L, Tq]   - so Q^T comes out of PE with A on partitions
  kT  = K[b].T       [A, N]    - guide and 1/sqrt(A) folded in on device
  v   = V[b]         [N, E]    - natural; AV contraction tiles n on partitions
  wqT = Wq.T         [L, A]
  msk = mask[b] u8   [Tq, N]
Softmax is computed without max-subtraction (att values are O(5), exp is safe
in f32, and softmax is shift-invariant); masked lanes are set to -200 before
exp so exp underflows to 0 and the fused accumulate row-sum is exact.
Normalization is applied to the [128, 512] output tile instead of the
[128, 2048] score tile (linearity of the AV matmul).
"""

import math

import numpy as np

import concourse.bass as bass
import concourse.mybir as mybir
import concourse.tile as tile
from concourse import bacc
from concourse.bass import ts
from concourse.bass_utils import run_bass_kernel_spmd
from concourse.masks import make_identity

B, TQ, N = 8, 1024, 2048
L, A, E = 1024, 128, 512
TEMP = 0.08
P = 128
LT = L // P    # 8 l-tiles (contraction tiles of the Q projection)
TT = TQ // P   # 8 t-tiles (rows of attention, 128 at a time)
NT = N // P    # 16 n-tiles (contraction tiles of the AV matmul)
NEG = -200.0   # masked logit value; exp(-200) underflows to exactly 0 in f32

F32 = mybir.dt.float32
F32R = mybir.dt.float32r
U8 = mybir.dt.uint8


def _r(ap):
    """Reinterpret an f32 AP as float32r for full-rate PE matmuls."""
    return ap.bitcast(F32R)


def build_nc():
    nc = bacc.Bacc("TRN2", target_bir_lowering=False, debug=False, num_devices=B)

    qT = nc.dram_tensor("qT", [L, TQ], F32R, kind="ExternalInput").ap()
    kT = nc.dram_tensor("kT", [A, N], F32, kind="ExternalInput").ap()
    v = nc.dram_tensor("v", [N, E], F32R, kind="ExternalInput").ap()
    wqT = nc.dram_tensor("wqT", [L, A], F32R, kind="ExternalInput").ap()
    stp = nc.dram_tensor("stp", [1, 1], F32, kind="ExternalInput").ap()
    msk = nc.dram_tensor("msk", [TQ, N], U8, kind="ExternalInput").ap()
    out = nc.dram_tensor("out", [TQ, E], F32, kind="ExternalOutput").ap()

    with tile.TileContext(nc) as tc:
        with (
            tc.tile_pool(name="const", bufs=1) as const,
            tc.tile_pool(name="setup", bufs=1) as setup,
            tc.tile_pool(name="mpool", bufs=3) as mpool,
            tc.tile_pool(name="spool", bufs=2) as spool,
            tc.tile_pool(name="stpool", bufs=2) as stpool,
            tc.tile_pool(name="opool", bufs=3) as opool,
            tc.tile_pool(name="small", bufs=6) as small,
            tc.tile_pool(name="psA", bufs=1, space="PSUM") as psA,
            tc.tile_pool(name="psB", bufs=2, space="PSUM") as psB,
            tc.tile_pool(name="psO", bufs=2, space="PSUM") as psO,
        ):
            # ---- one-time setup ----
            ident = const.tile([P, P], F32)
            make_identity(nc, ident)
            identr = const.tile([P, P], F32R)
            nc.vector.tensor_copy(identr, ident)

            step_sb = const.tile([P, 1], F32)
            nc.gpsimd.dma_start(out=step_sb, in_=stp.to_broadcast((P, 1)))
            nstep = const.tile([P, 1], F32)
            nc.vector.tensor_scalar_mul(nstep, step_sb, -1.0)

            neg_tile = const.tile([P, N], F32)
            nc.vector.memset(neg_tile, NEG)

            # guide row, replicated across all 128 partitions:
            #   guide[n] = exp(-((n+1)/N - step)^2 / TEMP - 0.5*ln(A))
            # (the 1/sqrt(A) attention norm is folded into the bias)
            pos = setup.tile([P, N], F32)
            nc.gpsimd.iota(
                pos,
                pattern=[[1, N]],
                base=1,
                channel_multiplier=0,
                allow_small_or_imprecise_dtypes=True,
            )
            z = setup.tile([P, N], F32)
            nc.scalar.activation(
                out=z,
                in_=pos,
                func=mybir.ActivationFunctionType.Square,
                bias=nstep,
                scale=1.0 / N,
            )
            gbias = const.tile([P, 1], F32)
            nc.vector.memset(gbias, -0.5 * math.log(A))
            guide = setup.tile([P, N], F32)
            nc.scalar.activation(
                out=guide,
                in_=z,
                func=mybir.ActivationFunctionType.Exp,
                scale=-1.0 / TEMP,
                bias=gbias,
            )

            # K^T with guide folded in: ksc[a, n] = K[n, a] * guide[n]
            kt_sb = setup.tile([P, N], F32)
            nc.sync.dma_start(out=kt_sb, in_=kT)
            ksc = const.tile([P, N], F32R)
            nc.vector.tensor_mul(ksc, kt_sb, guide)

            # Wq^T tiles: wq_sb[p, lt, a] = Wq[a, lt*128+p]
            wq_sb = const.tile([P, LT, A], F32R)
            nc.sync.dma_start(out=wq_sb, in_=wqT.rearrange("(lt p) a -> p lt a", p=P))

            # query^T tiles: qt_in[p, lt, t] = query[t, lt*128+p]
            # (split the DMA by l-tile so Q-projection matmuls start early)
            qt_in = setup.tile([P, LT, TQ], F32R)
            for lt in range(LT):
                nc.sync.dma_start(
                    out=qt_in[:, lt, :],
                    in_=qT.rearrange("(lt p) t -> p lt t", p=P)[:, lt, :],
                )

            # Q^T[a, t] = sum_l Wq[a, l] * query[t, l]
            qt = const.tile([P, TQ], F32R)
            for th in range(TQ // 512):
                ps_qt = psB.tile([P, 512], F32, tag="psb")
                for lt in range(LT):
                    nc.tensor.matmul(
                        ps_qt,
                        wq_sb[:, lt, :],
                        qt_in[:, lt, ts(th, 512)],
                        start=(lt == 0),
                        stop=(lt == LT - 1),
                    )
                nc.scalar.copy(qt[:, ts(th, 512)], ps_qt)

            # V tiles: v_sb[p, nt, e] = V[nt*128+p, e]
            v_sb = const.tile([P, NT, E], F32R)
            for vh in range(4):
                nc.sync.dma_start(
                    out=v_sb[:, ts(vh, NT // 4), :],
                    in_=v.rearrange("(nt p) e -> p nt e", p=P)[:, ts(vh, NT // 4), :],
                )

            # ---- main loop over 128-row tiles of Tq ----
            for ti in range(TT):
                mk = mpool.tile([P, N], U8)
                nc.sync.dma_start(out=mk, in_=msk[ts(ti, P), :])

                # att[t, n] = Q^T[:, t-tile].T @ ksc  (guide and norm already in ksc)
                att = psA.tile([P, N], F32)
                for j in range(N // 512):
                    nc.tensor.matmul(
                        att[:, ts(j, 512)],
                        qt[:, ts(ti, P)],
                        ksc[:, ts(j, 512)],
                        start=True,
                        stop=True,
                    )

                # masked lanes -> -200 (exp underflows to 0)
                nc.vector.copy_predicated(out=att, mask=mk, data=neg_tile)

                # s = exp(att), rs = row-sum(s) fused on the scalar engine
                s = spool.tile([P, N], F32R)
                rs = small.tile([P, 1], F32)
                nc.scalar.activation(
                    out=s,
                    in_=att,
                    func=mybir.ActivationFunctionType.Exp,
                    accum_out=rs,
                )
                rc = small.tile([P, 1], F32)
                nc.vector.reciprocal(rc, rs)

                # s^T in 128x128 blocks via PE transpose (4 blocks per PSUM bank)
                st = stpool.tile([P, N], F32R)
                for g in range(4):
                    ps_tr = psB.tile([P, 512], F32R, tag="psb")
                    for j in range(4):
                        nc.tensor.transpose(
                            ps_tr[:, ts(j, P)],
                            s[:, ts(g * 4 + j, P)],
                            identr,
                        )
                    if g % 2 == 0:
                        nc.scalar.copy(st[:, ts(g, 512)], ps_tr)
                    else:
                        nc.vector.tensor_copy(st[:, ts(g, 512)], ps_tr)

                # out[t, e] = sum_n s[t, n] * V[n, e], accumulated over 16 n-tiles
                ot = psO.tile([P, E], F32)
                for nt in range(NT):
                    nc.tensor.matmul(
                        ot,
                        st[:, ts(nt, P)],
                        v_sb[:, nt, :],
                        start=(nt == 0),
                        stop=(nt == NT - 1),
                    )

                # normalize on the narrow output tile and store
                ob = opool.tile([P, E], F32)
                nc.scalar.activation(
                    out=ob,
                    in_=ot,
                    func=mybir.ActivationFunctionType.Copy,
                    scale=rc,
                )
                nc.gpsimd.dma_start(out=out[ts(ti, P), :], in_=ob)

    nc.compile()
    return nc


def make_in_maps(query, K, V, Wq, step, mask):
    query = np.asarray(query, dtype=np.float32)
    K = np.asarray(K, dtype=np.float32)
    V = np.asarray(V, dtype=np.float32)
    Wq = np.asarray(Wq, dtype=np.float32)
    step = np.asarray(step, dtype=np.float32)
    mask = np.asarray(mask)
    if mask.dtype != np.uint8:
        mask = mask.astype(np.uint8)

    wqT = np.ascontiguousarray(Wq.T)
    stp = step.reshape(1, 1)
    in_maps = []
    for b in range(B):
        in_maps.append(
            {
                "qT": np.ascontiguousarray(query[b].T),
                "kT": np.ascontiguousarray(K[b].T),
                "v": np.ascontiguousarray(V[b]),
                "wqT": wqT,
                "stp": stp,
                "msk": mask[b],
            }
        )
    return in_maps


def kernel(query, K, V, Wq, step, mask):
    nc = build_nc()
    in_maps = make_in_maps(query, K, V, Wq, step, mask)
    res = run_bass_kernel_spmd(nc, in_maps, core_ids=list(range(B)))
    return np.stack([res.results[b]["out"] for b in range(B)], axis=0)


if __name__ == "__main__":
    rng = np.random.default_rng(0)
    inputs = {
        "query": rng.standard_normal((B, TQ, L), dtype=np.float32),
        "K": rng.standard_normal((B, N, A), dtype=np.float32),
        "V": rng.standard_normal((B, N, E), dtype=np.float32),
        "Wq": rng.standard_normal((A, L), dtype=np.float32) / math.sqrt(L),
        "step": rng.random((1,), dtype=np.float32),
        "mask": rng.integers(0, 2, size=(B, TQ, N)) > 0,
    }
    out = kernel(**inputs)
    print(out.shape, out.dtype)


# revision 9
# speedup vs baseline: 2.7458x; 1.6752x over previous
"""Trainium2 Bass kernel for batched cross-attention with gaussian guide mask.

Reference computation (per batch b):
  Q   = query @ Wq.T                      # [Tq, A]
  att = (Q @ K.T / sqrt(A)) * guide       # guide[n] = exp(-(step-(n+1)/N)^2/TEMP)
  att = where(mask, -inf, att)
  out = softmax(att, axis=-1) @ V         # [Tq, E]

Sharding: data-parallel over batch. Core b handles batch b (B == 8 == n_cores).

Device-side layout choices (host does layout-only prep: transposes/casts):
  qT  = query[b].T   [L, Tq]   - so Q^T comes out of PE with A on partitions
  kT  = K[b].T       [A, N]    - guide and 1/sqrt(A) folded in on device
  v   = V[b]         [N, E]    - natural; AV contraction tiles n on partitions
  wqT = Wq.T         [L, A]
  msk = mask[b] u8   [Tq, N]
Softmax is computed without max-subtraction (att values are O(5), exp is safe
in f32, and softmax is shift-invariant); masked lanes are set to -200 before
exp so exp underflows to 0 and the fused accumulate row-sum is exact.
Normalization is applied to the [128, 512] output tile instead of the
[128, 2048] score tile (linearity of the AV matmul).
"""

import math

import ml_dtypes
import numpy as np

import concourse.bass as bass
import concourse.mybir as mybir
import concourse.tile as tile
from concourse import bacc
from concourse.bass import ts
from concourse.bass_utils import run_bass_kernel_spmd
from concourse.masks import make_identity

B, TQ, N = 8, 1024, 2048
L, A, E = 1024, 128, 512
TEMP = 0.08
P = 128
LT = L // P    # 8 l-tiles (contraction tiles of the Q projection)
TT = TQ // P   # 8 t-tiles (rows of attention, 128 at a time)
NT = N // P    # 16 n-tiles (contraction tiles of the AV matmul)
NEG = -200.0   # masked logit value; exp(-200) underflows to exactly 0 in f32

F32 = mybir.dt.float32
F32R = mybir.dt.float32r
F16 = mybir.dt.float16
U8 = mybir.dt.uint8


def build_nc():
    nc = bacc.Bacc("TRN2", target_bir_lowering=False, debug=False, num_devices=B)

    qT = nc.dram_tensor("qT", [L, TQ], F32R, kind="ExternalInput").ap()
    kT = nc.dram_tensor("kT", [A, N], F32, kind="ExternalInput").ap()
    v = nc.dram_tensor("v", [N, E], F16, kind="ExternalInput").ap()
    wqT = nc.dram_tensor("wqT", [L, A], F32R, kind="ExternalInput").ap()
    stp = nc.dram_tensor("stp", [1, 1], F32, kind="ExternalInput").ap()
    msk = nc.dram_tensor("msk", [TQ, N], U8, kind="ExternalInput").ap()
    out = nc.dram_tensor("out", [TQ, E], F32, kind="ExternalOutput").ap()

    with tile.TileContext(nc) as tc:
        with (
            tc.tile_pool(name="const", bufs=1) as const,
            tc.tile_pool(name="setup", bufs=1) as setup,
            tc.tile_pool(name="mpool", bufs=3) as mpool,
            tc.tile_pool(name="spool", bufs=2) as spool,
            tc.tile_pool(name="stpool", bufs=2) as stpool,
            tc.tile_pool(name="opool", bufs=3) as opool,
            tc.tile_pool(name="small", bufs=6) as small,
            tc.tile_pool(name="psA", bufs=2, space="PSUM") as psA,
            tc.tile_pool(name="psB", bufs=2, space="PSUM") as psB,
            tc.tile_pool(name="psO", bufs=2, space="PSUM") as psO,
        ):
            # ---- one-time setup ----
            ident = const.tile([P, P], F32)
            make_identity(nc, ident)
            identh = const.tile([P, P], F16)
            nc.vector.tensor_copy(identh, ident)

            step_sb = const.tile([P, 1], F32)
            nc.gpsimd.dma_start(out=step_sb, in_=stp.to_broadcast((P, 1)))
            nstep = const.tile([P, 1], F32)
            nc.vector.tensor_scalar_mul(nstep, step_sb, -1.0)

            neg_tile = const.tile([P, N], F32)
            nc.vector.memset(neg_tile, NEG)

            # guide row, replicated across all 128 partitions:
            #   guide[n] = exp(-((n+1)/N - step)^2 / TEMP - 0.5*ln(A))
            # (the 1/sqrt(A) attention norm is folded into the bias)
            pos = setup.tile([P, N], F32)
            nc.gpsimd.iota(
                pos,
                pattern=[[1, N]],
                base=1,
                channel_multiplier=0,
                allow_small_or_imprecise_dtypes=True,
            )
            z = setup.tile([P, N], F32)
            nc.scalar.activation(
                out=z,
                in_=pos,
                func=mybir.ActivationFunctionType.Square,
                bias=nstep,
                scale=1.0 / N,
            )
            gbias = const.tile([P, 1], F32)
            nc.vector.memset(gbias, -0.5 * math.log(A))
            guide = setup.tile([P, N], F32)
            nc.scalar.activation(
                out=guide,
                in_=z,
                func=mybir.ActivationFunctionType.Exp,
                scale=-1.0 / TEMP,
                bias=gbias,
            )

            # K^T with guide folded in: ksc[a, n] = K[n, a] * guide[n]
            kt_sb = setup.tile([P, N], F32)
            nc.sync.dma_start(out=kt_sb, in_=kT)
            ksc = const.tile([P, N], F32R)
            nc.vector.tensor_mul(ksc, kt_sb, guide)

            # Wq^T tiles: wq_sb[p, lt, a] = Wq[a, lt*128+p]
            wq_sb = const.tile([P, LT, A], F32R)
            nc.sync.dma_start(out=wq_sb, in_=wqT.rearrange("(lt p) a -> p lt a", p=P))

            # query^T tiles: qt_in[p, lt, t] = query[t, lt*128+p]
            # (split the DMA by l-tile so Q-projection matmuls start early)
            qt_in = setup.tile([P, LT, TQ], F32R)
            for lt in range(LT):
                nc.sync.dma_start(
                    out=qt_in[:, lt, :],
                    in_=qT.rearrange("(lt p) t -> p lt t", p=P)[:, lt, :],
                )

            # Q^T[a, t] = sum_l Wq[a, l] * query[t, l]
            qt = const.tile([P, TQ], F32R)
            for th in range(TQ // 512):
                ps_qt = psO.tile([P, 512], F32, tag="pso")
                for lt in range(LT):
                    nc.tensor.matmul(
                        ps_qt,
                        wq_sb[:, lt, :],
                        qt_in[:, lt, ts(th, 512)],
                        start=(lt == 0),
                        stop=(lt == LT - 1),
                    )
                nc.scalar.copy(qt[:, ts(th, 512)], ps_qt)

            # V tiles: v_sb[p, nt, e] = V[nt*128+p, e]
            v_sb = const.tile([P, NT, E], F16)
            for vh in range(4):
                nc.sync.dma_start(
                    out=v_sb[:, ts(vh, NT // 4), :],
                    in_=v.rearrange("(nt p) e -> p nt e", p=P)[:, ts(vh, NT // 4), :],
                )

            # ---- main loop over 128-row tiles of Tq ----
            H = N // 2  # att is processed in two 1024-wide halves (2 PSUM banks each)
            for ti in range(TT):
                mk = mpool.tile([P, N], U8)
                nc.sync.dma_start(out=mk, in_=msk[ts(ti, P), :])

                s = spool.tile([P, N], F16)
                rs2 = small.tile([P, 2], F32)
                for h in range(2):
                    # att[t, n] = Q^T[:, t-tile].T @ ksc  (guide and norm in ksc)
                    att = psA.tile([P, H], F32, tag="att")
                    for j in range(H // 512):
                        nc.tensor.matmul(
                            att[:, ts(j, 512)],
                            qt[:, ts(ti, P)],
                            ksc[:, ts(h * 2 + j, 512)],
                            start=True,
                            stop=True,
                        )
                    # masked lanes -> -200 (exp underflows to 0)
                    nc.vector.copy_predicated(
                        out=att, mask=mk[:, ts(h, H)], data=neg_tile[:, ts(h, H)]
                    )
                    # s = exp(att) in bf16; rs = f32 row-sum fused on ScalarE
                    nc.scalar.activation(
                        out=s[:, ts(h, H)],
                        in_=att,
                        func=mybir.ActivationFunctionType.Exp,
                        accum_out=rs2[:, h : h + 1],
                    )

                rs = small.tile([P, 1], F32)
                nc.vector.tensor_reduce(
                    out=rs, in_=rs2, axis=mybir.AxisListType.X, op=mybir.AluOpType.add
                )
                rc = small.tile([P, 1], F32)
                nc.vector.reciprocal(rc, rs)

                # s^T in 128x128 blocks via PE transpose (8 blocks per PSUM bank)
                st = stpool.tile([P, N], F16)
                for g in range(2):
                    ps_tr = psB.tile([P, 1024], F16, tag="psb")
                    for j in range(8):
                        nc.tensor.transpose(
                            ps_tr[:, ts(j, P)],
                            s[:, ts(g * 8 + j, P)],
                            identh,
                        )
                    if g % 2 == 0:
                        nc.scalar.copy(st[:, ts(g, 1024)], ps_tr)
                    else:
                        nc.vector.tensor_copy(st[:, ts(g, 1024)], ps_tr)

                # out[t, e] = sum_n s[t, n] * V[n, e], accumulated over 16 n-tiles
                ot = psO.tile([P, E], F32, tag="pso")
                for nt in range(NT):
                    nc.tensor.matmul(
                        ot,
                        st[:, ts(nt, P)],
                        v_sb[:, nt, :],
                        start=(nt == 0),
                        stop=(nt == NT - 1),
                    )

                # normalize on the narrow output tile and store
                ob = opool.tile([P, E], F32)
                nc.scalar.activation(
                    out=ob,
                    in_=ot,
                    func=mybir.ActivationFunctionType.Copy,
                    scale=rc,
                )
                nc.gpsimd.dma_start(out=out[ts(ti, P), :], in_=ob)

    nc.compile()
    return nc


def make_in_maps(query, K, V, Wq, step, mask):
    query = np.asarray(query, dtype=np.float32)
    K = np.asarray(K, dtype=np.float32)
    V = np.asarray(V, dtype=np.float32)
    Wq = np.asarray(Wq, dtype=np.float32)
    step = np.asarray(step, dtype=np.float32)
    mask = np.asarray(mask)
    if mask.dtype != np.uint8:
        mask = mask.astype(np.uint8)

    wqT = np.ascontiguousarray(Wq.T)
    stp = step.reshape(1, 1)
    in_maps = []
    for b in range(B):
        in_maps.append(
            {
                "qT": np.ascontiguousarray(query[b].T),
                "kT": np.ascontiguousarray(K[b].T),
                "v": np.ascontiguousarray(V[b]).astype(np.float16),
                "wqT": wqT,
                "stp": stp,
                "msk": mask[b],
            }
        )
    return in_maps


def kernel(query, K, V, Wq, step, mask):
    nc = build_nc()
    in_maps = make_in_maps(query, K, V, Wq, step, mask)
    res = run_bass_kernel_spmd(nc, in_maps, core_ids=list(range(B)))
    return np.stack([res.results[b]["out"] for b in range(B)], axis=0)


if __name__ == "__main__":
    rng = np.random.default_rng(0)
    inputs = {
        "query": rng.standard_normal((B, TQ, L), dtype=np.float32),
        "K": rng.standard_normal((B, N, A), dtype=np.float32),
        "V": rng.standard_normal((B, N, E), dtype=np.float32),
        "Wq": rng.standard_normal((A, L), dtype=np.float32) / math.sqrt(L),
        "step": rng.random((1,), dtype=np.float32),
        "mask": rng.integers(0, 2, size=(B, TQ, N)) > 0,
    }
    out = kernel(**inputs)
    print(out.shape, out.dtype)
